# revision 12
# baseline (speedup 1.0000x reference)
"""Trainium2 Bass kernel for 2-layer GCN (nn_GCN_22866405884174).

Strategy (8 NeuronCores, dst-node sharding):
  out = A @ relu((A @ x) @ W1 + b1) @ W2 + b2   with A = D^-1/2 (Adj+I) D^-1/2
  (linear layers commute with aggregation, so each layer is: gather table
  rows by edge src + scatter-add by edge dst, then a small dense matmul).

  - Nodes sharded contiguously: core c owns dst nodes [c*12500, (c+1)*12500).
  - Layer 1: messages are HOST-materialized in slot order (the edge indices
    are known host-side), so the device just streams them contiguously via
    HWDGE — zero Q7 descriptor-generation work.
  - Layer 2: SWDGE dma_gather from the AllGathered hidden table (1024-index
    chunks, 4 queues = 4 Q7 core pairs generating descriptors in parallel).
  - Scatter-add: PE matmul msg[slots,feat]^T @ S[slots,128dst] accumulated
    into a full PSUM bank [128, 512] per 8-window block. S matrices are
    exact one-hot fp8 built on host (shared by both layers).
  - Between layers: AllGather of the bf16 hidden table across the 8 cores.
"""

import numpy as np
import ml_dtypes

# ---------------- problem constants (hardcoded per contract) ----------------
N = 100000
E = 1600000
F_IN = 128
HID = 64
OUT_D = 10

NCORES = 8
NPC = N // NCORES           # 12500 nodes per core
SH = 12544                  # padded shard rows (98 * 128)
NTOT = SH * NCORES          # 100352
SEC = 25088                 # table section rows (2 shards, < int16 range)
NSEC = 4
WDST = 64                   # dst window width
NWIN = (NPC + WDST - 1) // WDST   # 196 (last window = 20 dst)
WB = 8                      # windows per block
NBLK = (NWIN + WB - 1) // WB      # 25 (last block = 4 windows)
BCOLS = WB * WDST           # 512 psum cols per block
NG = SH // 128              # 98 node groups per shard
SENTINEL = 12500            # zero pad row (same local idx in every section)
CHUNK = 2048                # L2 gather chunk (multi-packet mode)
PIECE = 32                  # L1 stream piece (groups per dma_start)

_CACHE = {}


# ============================ host preprocessing ============================

def _host_prep(edge_index):
    src = np.asarray(edge_index[0]).astype(np.int64)
    dst = np.asarray(edge_index[1]).astype(np.int64)
    loops = np.arange(N, dtype=np.int64)
    src = np.concatenate([src, loops])
    dst = np.concatenate([dst, loops])
    deg = np.bincount(dst, minlength=N).astype(np.float32)
    dinv = 1.0 / np.sqrt(deg)

    srow = (src // NPC) * SH + (src % NPC)
    core = dst // NPC
    dloc = dst % NPC
    win = dloc // WDST
    sec = srow // SEC

    cellid = (core * NWIN + win) * NSEC + sec
    counts = np.bincount(cellid, minlength=NCORES * NWIN * NSEC).reshape(NCORES, NWIN, NSEC)
    n_cell = counts.max(axis=0)
    n_cell = np.maximum(((n_cell + 15) // 16) * 16, 128)   # 16-aligned, >= 128

    # ---- schedule: section-major slot streams, (block,sec) runs 128-aligned ----
    blocks = [[None] * NSEC for _ in range(NBLK)]
    sec_len = [0] * NSEC
    for s in range(NSEC):
        off = 0
        for b in range(NBLK):
            wlo, whi = b * WB, min(NWIN, (b + 1) * WB)
            cells = [int(n_cell[w, s]) for w in range(wlo, whi)]
            nbs = sum(cells)
            run = ((nbs + 127) // 128) * 128
            ngrp = run // 128
            bounds = np.cumsum([0] + cells)
            groups = []
            for j in range(ngrp):
                gslot = off + j * 128
                wi = int(np.searchsorted(bounds, j * 128, side="right") - 1)
                wi = min(wi, len(cells) - 1)
                base = min(wi * WDST, BCOLS - 128)
                groups.append((gslot, base))
            blocks[b][s] = {
                "cells": cells, "nbs": nbs, "run": run, "ngrp": ngrp,
                "groups": groups, "soff": off,
            }
            off += run
        sec_len[s] = off

    # S group offsets in (b, s, j) order
    TG = 0
    for b in range(NBLK):
        for s in range(NSEC):
            blocks[b][s]["gi"] = TG
            TG += blocks[b][s]["ngrp"]

    # idx tensor: queue-pinned chunk layout. Chunk (s,k) is assigned to SWDGE
    # queue q (round-robin in device issue order); its idx columns live at
    # qcol[(s,k)] and only in partitions [32q, 32q+32) (the worker core pair).
    chunks_tmp = [[] for _ in range(NSEC)]
    for s in range(NSEC):
        rem = sec_len[s]
        while rem > 0:
            chunks_tmp[s].append(min(CHUNK, rem))
            rem -= min(CHUNK, rem)
    qof = [0, 0, 0, 0]
    qmap = {}
    qn = 0
    kmax_t = max(len(chunks_tmp[s]) for s in range(NSEC))
    for k in range(kmax_t):
        for s in range(NSEC):
            if k >= len(chunks_tmp[s]):
                continue
            qmap[(s, k)] = (qn, qof[qn])
            qof[qn] += chunks_tmp[s][k] // 16
            qn = (qn + 1) % 4
    CIDX = max(qof)

    # slot-group offset of each section stream within the whole msg stream
    sec_goff = [0] * NSEC
    TOTG = 0
    for s in range(NSEC):
        sec_goff[s] = TOTG
        TOTG += sec_len[s] // 128

    chunks = chunks_tmp                  # per section: chunk sizes

    sort_key = (sec + NSEC * (win + NWIN * core))
    order = np.lexsort((dloc, sort_key))
    srow_s = srow[order]
    dloc_s = dloc[order]
    key_s = sort_key[order]

    idx_all = np.zeros((NCORES, 128, CIDX), dtype=np.int16)
    sval_all = np.zeros((NCORES, TG, 128, 128), dtype=ml_dtypes.float8_e4m3)
    dinv_gt = np.zeros((NCORES, 128, NG), dtype=np.float32)
    # global table row per slot (per core), for host-side L1 message gather
    stream_glob = np.zeros((NCORES, TG * 128), dtype=np.int64)

    cw_starts = np.searchsorted(key_s, np.arange(NCORES * NWIN * NSEC + 1))
    for c in range(NCORES):
        s_g = []
        s_p = []
        s_d = []
        for s in range(NSEC):
            stream = np.full(sec_len[s], SENTINEL, dtype=np.int64)
            dcol_st = np.full(sec_len[s], -1, dtype=np.int64)
            for b in range(NBLK):
                info = blocks[b][s]
                off = info["soff"]
                wlo, whi = b * WB, min(NWIN, (b + 1) * WB)
                for wi, w in enumerate(range(wlo, whi)):
                    cid = (c * NWIN + w) * NSEC + s
                    a, e = cw_starts[cid], cw_starts[cid + 1]
                    cnt = e - a
                    stream[off:off + cnt] = srow_s[a:e] - s * SEC
                    dcol_st[off:off + cnt] = dloc_s[a:e] - b * BCOLS
                    off += info["cells"][wi]
                # S coords for this (b, s): slots [soff, soff+run)
                t0, t1 = info["soff"], info["soff"] + info["run"]
                t = np.arange(t0, t1)
                dc = dcol_st[t0:t1]
                real = dc >= 0
                j = (t - t0) // 128
                bases = np.array([g[1] for g in info["groups"]], dtype=np.int64)
                scol = dc - bases[j]
                if real.any():
                    assert scol[real].min() >= 0 and scol[real].max() < 128
                s_g.append(info["gi"] + j[real])
                s_p.append((t[real] - t0) % 128)
                s_d.append(scol[real])
            st16 = stream.astype(np.int16).reshape(sec_len[s] // 16, 16).T
            for k in range(len(chunks[s])):
                q, col = qmap[(s, k)]
                ncol = chunks[s][k] // 16
                blk16 = st16[:, k * (CHUNK // 16):k * (CHUNK // 16) + ncol]
                idx_all[c, 32 * q:32 * q + 16, col:col + ncol] = blk16
                idx_all[c, 32 * q + 16:32 * q + 32, col:col + ncol] = blk16
            for b in range(NBLK):
                info = blocks[b][s]
                gi, soff, run = info["gi"], info["soff"], info["run"]
                stream_glob[c, gi * 128:gi * 128 + run] = stream[soff:soff + run] + s * SEC
        sval_all[c, np.concatenate(s_g), np.concatenate(s_p),
                 np.concatenate(s_d)] = ml_dtypes.float8_e4m3(1.0)

        dpad = np.zeros(SH, dtype=np.float32)
        dpad[:NPC] = dinv[c * NPC:(c + 1) * NPC]
        dinv_gt[c] = dpad.reshape(NG, 128).T

    sched = {
        "blocks": blocks, "chunks": chunks, "qmap": qmap,
        "sec_len": sec_len, "CIDX": CIDX, "TG": TG,
    }
    return sched, idx_all, sval_all, dinv_gt, dinv, stream_glob


# ============================ device program ============================

def build_program(sched):
    import concourse.bass as bass
    import concourse.bacc as bacc
    import concourse.tile as tile
    import concourse.mybir as mybir

    CIDX = sched["CIDX"]
    TG = sched["TG"]
    blocks = sched["blocks"]
    chunks = sched["chunks"]
    qmap = sched["qmap"]
    GBLK_MAX = max(sum(blocks[b][s]["ngrp"] for s in range(NSEC)) for b in range(NBLK))

    nc = bacc.Bacc(None, target_bir_lowering=False, debug=False, num_swdge_queues=4,
                   dynamic_dma_scratch_size=32768)
    f32 = mybir.dt.float32
    bf16 = mybir.dt.bfloat16
    fp8 = mybir.dt.float8e4
    i16 = mybir.dt.int16

    M1D = nc.dram_tensor("M1D", [128, TG, F_IN], bf16, kind="ExternalInput")
    IDX = nc.dram_tensor("IDX", [128, CIDX], i16, kind="ExternalInput")
    SVAL = nc.dram_tensor("SVAL", [TG, 128, 128], fp8, kind="ExternalInput")
    DINV = nc.dram_tensor("DINV", [128, NG], f32, kind="ExternalInput")
    W1T = nc.dram_tensor("W1T", [F_IN, HID], bf16, kind="ExternalInput")
    B1T = nc.dram_tensor("B1T", [128, HID], f32, kind="ExternalInput")
    W2T = nc.dram_tensor("W2T", [HID, OUT_D], bf16, kind="ExternalInput")
    B2T = nc.dram_tensor("B2T", [128, OUT_D], f32, kind="ExternalInput")
    OUTE = nc.dram_tensor("OUTE", [SH, OUT_D], f32, kind="ExternalOutput")

    t_local = nc.dram_tensor("t_local", [SH, F_IN], bf16)
    t_full = nc.dram_tensor("t_full", [NTOT, F_IN], bf16, addr_space="Shared")

    with tile.TileContext(nc) as tc:
        with (
            tc.tile_pool(name="resident", bufs=1) as rpool,
            tc.tile_pool(name="msg", bufs=10) as mpool,
            tc.tile_pool(name="msgb", bufs=3) as bpool,
            tc.tile_pool(name="sv", bufs=3) as spool,
            tc.tile_pool(name="post", bufs=4) as ppool,
            tc.tile_pool(name="psum", bufs=3, space="PSUM") as psum_pool,
            tc.tile_pool(name="psum2", bufs=4, space="PSUM") as psum_pool2,
        ):
            idx_t = rpool.tile([128, CIDX], i16)
            nc.sync.dma_start(idx_t[:], IDX[:])
            dinv_t = rpool.tile([128, NG], f32)
            nc.sync.dma_start(dinv_t[:], DINV[:])
            w1_t = rpool.tile([F_IN, HID], bf16)
            nc.sync.dma_start(w1_t[:], W1T[:])
            b1_t = rpool.tile([128, HID], f32)
            nc.sync.dma_start(b1_t[:], B1T[:])
            w2_t = rpool.tile([HID, OUT_D], bf16)
            nc.sync.dma_start(w2_t[:], W2T[:])
            b2_t = rpool.tile([128, OUT_D], f32)
            nc.sync.dma_start(b2_t[:], B2T[:])

            agg1 = rpool.tile([128, SH], bf16)
            agg2 = rpool.tile([HID, SH], bf16)

            def scatter_layer(table):
                # layer 2: SWDGE gathers, round-robin over 4 queues
                msg_tiles = {}
                kmax = max(len(chunks[s]) for s in range(NSEC))
                for k in range(kmax):
                    for s in range(NSEC):
                        if k >= len(chunks[s]):
                            continue
                        csz = chunks[s][k]
                        ng = (csz + 127) // 128
                        qn, col = qmap[(s, k)]
                        msg = mpool.tile([128, CHUNK // 128, F_IN], bf16, tag="msg")
                        nc.gpsimd.dma_gather(
                            msg[:, :ng, :],
                            table[s * SEC:(s + 1) * SEC, :],
                            idx_t[:, col:col + csz // 16],
                            csz,
                            csz,
                            F_IN,
                            single_packet=False,
                            queue_num=qn,
                        )
                        msg_tiles[(s, k)] = msg

                def access(s, gslot, gi):
                    return msg_tiles[(s, gslot // CHUNK)][:, (gslot % CHUNK) // 128, :]
                return access

            def stream_layer():
                # layer 1: host-materialized messages in block-consumption
                # (gi) order -- strictly sequential contiguous HWDGE stream
                msg_tiles = {}
                for p0 in range(0, TG, PIECE):
                    ng = min(PIECE, TG - p0)
                    msg = bpool.tile([128, PIECE, F_IN], bf16, tag="msgb")
                    nc.scalar.dma_start(msg[:, :ng, :], M1D[:, p0:p0 + ng, :])
                    msg_tiles[p0 // PIECE] = msg

                def access(s, gslot, gi):
                    return msg_tiles[gi // PIECE][:, gi % PIECE, :]
                return access

            def consume_blocks(access, agg):
                for b in range(NBLK):
                    blo = b * BCOLS
                    blen = min(BCOLS, NPC - blo)
                    gblk = sum(blocks[b][s]["ngrp"] for s in range(NSEC))
                    g0 = blocks[b][0]["gi"]
                    s_t = spool.tile([128, GBLK_MAX, 128], fp8, tag="sval")
                    nc.sync.dma_start(
                        s_t[:, :gblk, :],
                        SVAL[g0:g0 + gblk].rearrange("g p w -> p g w"),
                    )
                    acc = psum_pool.tile([128, BCOLS], f32, tag="acc")
                    mm = 0
                    for s in range(NSEC):
                        info = blocks[b][s]
                        for (gslot, base) in info["groups"]:
                            nc.tensor.matmul(
                                acc[:, base:base + 128],
                                access(s, gslot, g0 + mm),
                                s_t[:, mm, :],
                                start=(mm == 0),
                                stop=(mm == gblk - 1),
                            )
                            mm += 1
                    nc.vector.tensor_copy(
                        agg[:, blo:blo + blen], acc[:agg.shape[0], :blen]
                    )
                nc.vector.memset(agg[:, NPC:SH], 0.0)

            # ---------------- layer 1 ----------------
            consume_blocks(stream_layer(), agg1)

            for g in range(NG):
                ph = psum_pool2.tile([128, HID], f32, tag="wout")
                nc.tensor.matmul(
                    ph[:], agg1[:, g * 128:(g + 1) * 128], w1_t[:],
                    start=True, stop=True,
                )
                tmp = ppool.tile([128, HID], f32, tag="tmp")
                nc.vector.tensor_scalar(
                    out=tmp[:], in0=ph[:], scalar1=dinv_t[:, g:g + 1],
                    scalar2=None, op0=mybir.AluOpType.mult,
                )
                nc.vector.tensor_tensor(
                    out=tmp[:], in0=tmp[:], in1=b1_t[:], op=mybir.AluOpType.add
                )
                t_out = ppool.tile([128, F_IN], bf16, tag="tout")
                nc.vector.memset(t_out[:, HID:], 0.0)
                nc.vector.tensor_scalar(
                    out=t_out[:, :HID], in0=tmp[:], scalar1=0.0,
                    scalar2=dinv_t[:, g:g + 1], op0=mybir.AluOpType.max,
                    op1=mybir.AluOpType.mult,
                )
                nc.scalar.dma_start(t_local[g * 128:(g + 1) * 128, :], t_out[:])

            nc.gpsimd.collective_compute(
                "AllGather",
                mybir.AluOpType.bypass,
                replica_groups=[list(range(NCORES))],
                ins=[t_local[:]],
                outs=[t_full[:]],
            )

            # ---------------- layer 2 ----------------
            consume_blocks(scatter_layer(t_full), agg2)

            for g in range(NG):
                po = psum_pool2.tile([128, OUT_D], f32, tag="wout")
                nc.tensor.matmul(
                    po[:], agg2[:, g * 128:(g + 1) * 128], w2_t[:],
                    start=True, stop=True,
                )
                ot = ppool.tile([128, OUT_D], f32, tag="ot")
                nc.vector.tensor_scalar(
                    out=ot[:], in0=po[:], scalar1=dinv_t[:, g:g + 1],
                    scalar2=None, op0=mybir.AluOpType.mult,
                )
                nc.vector.tensor_tensor(
                    out=ot[:], in0=ot[:], in1=b2_t[:], op=mybir.AluOpType.add
                )
                nc.sync.dma_start(OUTE[g * 128:(g + 1) * 128, :], ot[:])

    nc.compile()
    return nc


# ============================ entry point ============================

def prepare(x, edge_index, W1, b1, W2, b2):
    x = np.asarray(x, dtype=np.float32)
    W1 = np.asarray(W1, dtype=np.float32)
    b1 = np.asarray(b1, dtype=np.float32)
    W2 = np.asarray(W2, dtype=np.float32)
    b2 = np.asarray(b2, dtype=np.float32)

    sched, idx_all, sval_all, dinv_gt, dinv, stream_glob = _host_prep(edge_index)

    key = ("v5d", sched["CIDX"], sched["TG"])
    if key in _CACHE:
        nc = _CACHE[key]
    else:
        nc = build_program(sched)
        _CACHE[key] = nc

    xs = x * dinv[:, None]
    T1 = np.zeros((NTOT, F_IN), dtype=ml_dtypes.bfloat16)
    for c in range(NCORES):
        T1[c * SH:c * SH + NPC] = xs[c * NPC:(c + 1) * NPC].astype(ml_dtypes.bfloat16)

    TG = sched["TG"]
    b1_tile = np.tile(b1[None, :], (128, 1)).astype(np.float32)
    b2_tile = np.tile(b2[None, :], (128, 1)).astype(np.float32)

    in_maps = []
    for c in range(NCORES):
        m1 = T1[stream_glob[c]].reshape(TG, 128, F_IN).transpose(1, 0, 2)
        in_maps.append({
            "M1D": np.ascontiguousarray(m1),
            "IDX": np.ascontiguousarray(idx_all[c]),
            "SVAL": np.ascontiguousarray(sval_all[c]),
            "DINV": np.ascontiguousarray(dinv_gt[c]),
            "W1T": W1.astype(ml_dtypes.bfloat16),
            "B1T": b1_tile,
            "W2T": W2.astype(ml_dtypes.bfloat16),
            "B2T": b2_tile,
        })
    return nc, in_maps


def kernel(x, edge_index, W1, b1, W2, b2):
    from concourse.bass_utils import run_bass_kernel_spmd

    nc, in_maps = prepare(x, edge_index, W1, b1, W2, b2)
    r = run_bass_kernel_spmd(nc, in_maps, core_ids=list(range(NCORES)))
    out = np.empty((N, OUT_D), dtype=np.float32)
    for c in range(NCORES):
        out[c * NPC:(c + 1) * NPC] = r.results[c]["OUTE"][:NPC]
    return out


# revision 13
# speedup vs baseline: 1.4155x; 1.4155x over previous
"""Trainium2 Bass kernel for 2-layer GCN (nn_GCN_22866405884174).

Strategy (8 NeuronCores, dst-node sharding):
  out = A @ relu((A @ x) @ W1 + b1) @ W2 + b2   with A = D^-1/2 (Adj+I) D^-1/2
  (linear layers commute with aggregation, so each layer is: gather table
  rows by edge src + scatter-add by edge dst, then a small dense matmul).

  - Nodes sharded contiguously: core c owns dst nodes [c*12500, (c+1)*12500).
  - Layer 1: messages are HOST-materialized in slot order (the edge indices
    are known host-side), so the device just streams them contiguously via
    HWDGE — zero Q7 descriptor-generation work.
  - Layer 2: SWDGE dma_gather from the AllGathered hidden table (1024-index
    chunks, 4 queues = 4 Q7 core pairs generating descriptors in parallel).
  - Scatter-add: PE matmul msg[slots,feat]^T @ S[slots,128dst] accumulated
    into a full PSUM bank [128, 512] per 8-window block. S matrices are
    exact one-hot fp8 built on host (shared by both layers).
  - Between layers: AllGather of the bf16 hidden table across the 8 cores.
"""

import numpy as np
import ml_dtypes

# ---------------- problem constants (hardcoded per contract) ----------------
N = 100000
E = 1600000
F_IN = 128
HID = 64
OUT_D = 10

NCORES = 8
NPC = N // NCORES           # 12500 nodes per core
SH = 12544                  # padded shard rows (98 * 128)
NTOT = SH * NCORES          # 100352
SEC = 25088                 # table section rows (2 shards, < int16 range)
NSEC = 4
WDST = 64                   # dst window width
NWIN = (NPC + WDST - 1) // WDST   # 196 (last window = 20 dst)
WB = 8                      # windows per block
NBLK = (NWIN + WB - 1) // WB      # 25 (last block = 4 windows)
BCOLS = WB * WDST           # 512 psum cols per block
NG = SH // 128              # 98 node groups per shard
SENTINEL = 12500            # zero pad row (same local idx in every section)
CHUNK = 1024                # L2 gather chunk
PIECE = 32                  # L1 stream piece (groups per dma_start)

_CACHE = {}


# ============================ host preprocessing ============================

def _host_prep(edge_index):
    src = np.asarray(edge_index[0]).astype(np.int64)
    dst = np.asarray(edge_index[1]).astype(np.int64)
    loops = np.arange(N, dtype=np.int64)
    src = np.concatenate([src, loops])
    dst = np.concatenate([dst, loops])
    deg = np.bincount(dst, minlength=N).astype(np.float32)
    dinv = 1.0 / np.sqrt(deg)

    srow = (src // NPC) * SH + (src % NPC)
    core = dst // NPC
    dloc = dst % NPC
    win = dloc // WDST
    sec = srow // SEC

    cellid = (core * NWIN + win) * NSEC + sec
    counts = np.bincount(cellid, minlength=NCORES * NWIN * NSEC).reshape(NCORES, NWIN, NSEC)
    n_cell = counts.max(axis=0)
    n_cell = np.maximum(((n_cell + 15) // 16) * 16, 128)   # 16-aligned, >= 128

    # ---- schedule: section-major slot streams, (block,sec) runs 128-aligned ----
    blocks = [[None] * NSEC for _ in range(NBLK)]
    sec_len = [0] * NSEC
    for s in range(NSEC):
        off = 0
        for b in range(NBLK):
            wlo, whi = b * WB, min(NWIN, (b + 1) * WB)
            cells = [int(n_cell[w, s]) for w in range(wlo, whi)]
            nbs = sum(cells)
            run = ((nbs + 127) // 128) * 128
            ngrp = run // 128
            bounds = np.cumsum([0] + cells)
            groups = []
            for j in range(ngrp):
                gslot = off + j * 128
                wi = int(np.searchsorted(bounds, j * 128, side="right") - 1)
                wi = min(wi, len(cells) - 1)
                base = min(wi * WDST, BCOLS - 128)
                groups.append((gslot, base))
            blocks[b][s] = {
                "cells": cells, "nbs": nbs, "run": run, "ngrp": ngrp,
                "groups": groups, "soff": off,
            }
            off += run
        sec_len[s] = off

    # S group offsets in (b, s, j) order
    TG = 0
    for b in range(NBLK):
        for s in range(NSEC):
            blocks[b][s]["gi"] = TG
            TG += blocks[b][s]["ngrp"]

    # idx tensor: queue-pinned chunk layout. Chunk (s,k) is assigned to SWDGE
    # queue q (round-robin in device issue order); its idx columns live at
    # qcol[(s,k)] and only in partitions [32q, 32q+32) (the worker core pair).
    chunks_tmp = [[] for _ in range(NSEC)]
    for s in range(NSEC):
        rem = sec_len[s]
        while rem > 0:
            chunks_tmp[s].append(min(CHUNK, rem))
            rem -= min(CHUNK, rem)
    qof = [0, 0, 0, 0]
    qmap = {}
    qn = 0
    kmax_t = max(len(chunks_tmp[s]) for s in range(NSEC))
    for k in range(kmax_t):
        for s in range(NSEC):
            if k >= len(chunks_tmp[s]):
                continue
            qmap[(s, k)] = (qn, qof[qn])
            qof[qn] += chunks_tmp[s][k] // 16
            qn = (qn + 1) % 4
    CIDX = max(qof)

    # slot-group offset of each section stream within the whole msg stream
    sec_goff = [0] * NSEC
    TOTG = 0
    for s in range(NSEC):
        sec_goff[s] = TOTG
        TOTG += sec_len[s] // 128

    chunks = chunks_tmp                  # per section: chunk sizes

    sort_key = (sec + NSEC * (win + NWIN * core))
    order = np.lexsort((dloc, sort_key))
    srow_s = srow[order]
    dloc_s = dloc[order]
    key_s = sort_key[order]

    idx_all = np.zeros((NCORES, 128, CIDX), dtype=np.int16)
    sval_all = np.zeros((NCORES, 128, TG, 128), dtype=ml_dtypes.float8_e4m3)
    dinv_gt = np.zeros((NCORES, 128, NG), dtype=np.float32)
    # global table row per slot (per core), for host-side L1 message gather
    stream_glob = np.zeros((NCORES, TG * 128), dtype=np.int64)

    cw_starts = np.searchsorted(key_s, np.arange(NCORES * NWIN * NSEC + 1))
    for c in range(NCORES):
        s_g = []
        s_p = []
        s_d = []
        for s in range(NSEC):
            stream = np.full(sec_len[s], SENTINEL, dtype=np.int64)
            dcol_st = np.full(sec_len[s], -1, dtype=np.int64)
            for b in range(NBLK):
                info = blocks[b][s]
                off = info["soff"]
                wlo, whi = b * WB, min(NWIN, (b + 1) * WB)
                for wi, w in enumerate(range(wlo, whi)):
                    cid = (c * NWIN + w) * NSEC + s
                    a, e = cw_starts[cid], cw_starts[cid + 1]
                    cnt = e - a
                    stream[off:off + cnt] = srow_s[a:e] - s * SEC
                    dcol_st[off:off + cnt] = dloc_s[a:e] - b * BCOLS
                    off += info["cells"][wi]
                # S coords for this (b, s): slots [soff, soff+run)
                t0, t1 = info["soff"], info["soff"] + info["run"]
                t = np.arange(t0, t1)
                dc = dcol_st[t0:t1]
                real = dc >= 0
                j = (t - t0) // 128
                bases = np.array([g[1] for g in info["groups"]], dtype=np.int64)
                scol = dc - bases[j]
                if real.any():
                    assert scol[real].min() >= 0 and scol[real].max() < 128
                s_g.append(info["gi"] + j[real])
                s_p.append((t[real] - t0) % 128)
                s_d.append(scol[real])
            st16 = stream.astype(np.int16).reshape(sec_len[s] // 16, 16).T
            for k in range(len(chunks[s])):
                q, col = qmap[(s, k)]
                ncol = chunks[s][k] // 16
                blk16 = st16[:, k * (CHUNK // 16):k * (CHUNK // 16) + ncol]
                idx_all[c, 32 * q:32 * q + 16, col:col + ncol] = blk16
                idx_all[c, 32 * q + 16:32 * q + 32, col:col + ncol] = blk16
            for b in range(NBLK):
                info = blocks[b][s]
                gi, soff, run = info["gi"], info["soff"], info["run"]
                stream_glob[c, gi * 128:gi * 128 + run] = stream[soff:soff + run] + s * SEC
        sval_all[c, np.concatenate(s_p), np.concatenate(s_g),
                 np.concatenate(s_d)] = ml_dtypes.float8_e4m3(1.0)

        dpad = np.zeros(SH, dtype=np.float32)
        dpad[:NPC] = dinv[c * NPC:(c + 1) * NPC]
        dinv_gt[c] = dpad.reshape(NG, 128).T

    sched = {
        "blocks": blocks, "chunks": chunks, "qmap": qmap,
        "sec_len": sec_len, "CIDX": CIDX, "TG": TG,
    }
    return sched, idx_all, sval_all, dinv_gt, dinv, stream_glob


# ============================ device program ============================

def build_program(sched):
    import concourse.bass as bass
    import concourse.bacc as bacc
    import concourse.tile as tile
    import concourse.mybir as mybir

    CIDX = sched["CIDX"]
    TG = sched["TG"]
    blocks = sched["blocks"]
    chunks = sched["chunks"]
    qmap = sched["qmap"]
    GBLK_MAX = max(sum(blocks[b][s]["ngrp"] for s in range(NSEC)) for b in range(NBLK))

    nc = bacc.Bacc(None, target_bir_lowering=False, debug=False, num_swdge_queues=4,
                   dynamic_dma_scratch_size=32768)
    f32 = mybir.dt.float32
    bf16 = mybir.dt.bfloat16
    fp8 = mybir.dt.float8e4
    i16 = mybir.dt.int16

    M1D = nc.dram_tensor("M1D", [128, TG, F_IN], bf16, kind="ExternalInput")
    IDX = nc.dram_tensor("IDX", [128, CIDX], i16, kind="ExternalInput")
    SVAL = nc.dram_tensor("SVAL", [128, TG, 128], fp8, kind="ExternalInput")
    DINV = nc.dram_tensor("DINV", [128, NG], f32, kind="ExternalInput")
    W1T = nc.dram_tensor("W1T", [F_IN, HID], bf16, kind="ExternalInput")
    B1T = nc.dram_tensor("B1T", [128, HID], f32, kind="ExternalInput")
    W2T = nc.dram_tensor("W2T", [HID, OUT_D], bf16, kind="ExternalInput")
    B2T = nc.dram_tensor("B2T", [128, OUT_D], f32, kind="ExternalInput")
    OUTE = nc.dram_tensor("OUTE", [SH, OUT_D], f32, kind="ExternalOutput")

    t_local = nc.dram_tensor("t_local", [SH, F_IN], bf16)
    t_full = nc.dram_tensor("t_full", [NTOT, F_IN], bf16, addr_space="Shared")

    with tile.TileContext(nc) as tc:
        with (
            tc.tile_pool(name="resident", bufs=1) as rpool,
            tc.tile_pool(name="msg", bufs=16) as mpool,
            tc.tile_pool(name="msgb", bufs=4) as bpool,
            tc.tile_pool(name="sv", bufs=3) as spool,
            tc.tile_pool(name="post", bufs=4) as ppool,
            tc.tile_pool(name="psum", bufs=3, space="PSUM") as psum_pool,
            tc.tile_pool(name="psum2", bufs=4, space="PSUM") as psum_pool2,
        ):
            idx_t = rpool.tile([128, CIDX], i16)
            nc.sync.dma_start(idx_t[:], IDX[:])
            dinv_t = rpool.tile([128, NG], f32)
            nc.sync.dma_start(dinv_t[:], DINV[:])
            w1_t = rpool.tile([F_IN, HID], bf16)
            nc.sync.dma_start(w1_t[:], W1T[:])
            b1_t = rpool.tile([128, HID], f32)
            nc.sync.dma_start(b1_t[:], B1T[:])
            w2_t = rpool.tile([HID, OUT_D], bf16)
            nc.sync.dma_start(w2_t[:], W2T[:])
            b2_t = rpool.tile([128, OUT_D], f32)
            nc.sync.dma_start(b2_t[:], B2T[:])

            agg1 = rpool.tile([128, SH], bf16)
            agg2 = rpool.tile([HID, SH], bf16)

            def scatter_layer(table):
                # layer 2: SWDGE gathers, round-robin over 4 queues
                msg_tiles = {}
                kmax = max(len(chunks[s]) for s in range(NSEC))
                for k in range(kmax):
                    for s in range(NSEC):
                        if k >= len(chunks[s]):
                            continue
                        csz = chunks[s][k]
                        ng = (csz + 127) // 128
                        qn, col = qmap[(s, k)]
                        msg = mpool.tile([128, CHUNK // 128, F_IN], bf16, tag="msg")
                        nc.gpsimd.dma_gather(
                            msg[:, :ng, :],
                            table[s * SEC:(s + 1) * SEC, :],
                            idx_t[:, col:col + csz // 16],
                            csz,
                            csz,
                            F_IN,
                            single_packet=True,
                            queue_num=qn,
                        )
                        msg_tiles[(s, k)] = msg

                def access(s, gslot, gi):
                    return msg_tiles[(s, gslot // CHUNK)][:, (gslot % CHUNK) // 128, :]
                return access

            def stream_layer():
                # layer 1: host-materialized messages in block-consumption
                # (gi) order -- strictly sequential contiguous HWDGE stream
                msg_tiles = {}
                for p0 in range(0, TG, PIECE):
                    ng = min(PIECE, TG - p0)
                    msg = bpool.tile([128, PIECE, F_IN], bf16, tag="msgb")
                    nc.scalar.dma_start(msg[:, :ng, :], M1D[:, p0:p0 + ng, :])
                    msg_tiles[p0 // PIECE] = msg

                def access(s, gslot, gi):
                    return msg_tiles[gi // PIECE][:, gi % PIECE, :]
                return access

            def consume_blocks(access, agg):
                for b in range(NBLK):
                    blo = b * BCOLS
                    blen = min(BCOLS, NPC - blo)
                    gblk = sum(blocks[b][s]["ngrp"] for s in range(NSEC))
                    g0 = blocks[b][0]["gi"]
                    s_t = spool.tile([128, GBLK_MAX, 128], fp8, tag="sval")
                    nc.sync.dma_start(
                        s_t[:, :gblk, :], SVAL[:, g0:g0 + gblk, :]
                    )
                    acc = psum_pool.tile([128, BCOLS], f32, tag="acc")
                    mm = 0
                    for s in range(NSEC):
                        info = blocks[b][s]
                        for (gslot, base) in info["groups"]:
                            nc.tensor.matmul(
                                acc[:, base:base + 128],
                                access(s, gslot, g0 + mm),
                                s_t[:, mm, :],
                                start=(mm == 0),
                                stop=(mm == gblk - 1),
                            )
                            mm += 1
                    nc.vector.tensor_copy(
                        agg[:, blo:blo + blen], acc[:agg.shape[0], :blen]
                    )
                nc.vector.memset(agg[:, NPC:SH], 0.0)

            # ---------------- layer 1 ----------------
            consume_blocks(stream_layer(), agg1)

            for g in range(NG):
                ph = psum_pool2.tile([128, HID], f32, tag="wout")
                nc.tensor.matmul(
                    ph[:], agg1[:, g * 128:(g + 1) * 128], w1_t[:],
                    start=True, stop=True,
                )
                tmp = ppool.tile([128, HID], f32, tag="tmp")
                nc.vector.tensor_scalar(
                    out=tmp[:], in0=ph[:], scalar1=dinv_t[:, g:g + 1],
                    scalar2=None, op0=mybir.AluOpType.mult,
                )
                nc.vector.tensor_tensor(
                    out=tmp[:], in0=tmp[:], in1=b1_t[:], op=mybir.AluOpType.add
                )
                t_out = ppool.tile([128, F_IN], bf16, tag="tout")
                nc.vector.memset(t_out[:, HID:], 0.0)
                nc.vector.tensor_scalar(
                    out=t_out[:, :HID], in0=tmp[:], scalar1=0.0,
                    scalar2=dinv_t[:, g:g + 1], op0=mybir.AluOpType.max,
                    op1=mybir.AluOpType.mult,
                )
                nc.scalar.dma_start(t_local[g * 128:(g + 1) * 128, :], t_out[:])

            nc.gpsimd.collective_compute(
                "AllGather",
                mybir.AluOpType.bypass,
                replica_groups=[list(range(NCORES))],
                ins=[t_local[:]],
                outs=[t_full[:]],
            )

            # ---------------- layer 2 ----------------
            consume_blocks(scatter_layer(t_full), agg2)

            for g in range(NG):
                po = psum_pool2.tile([128, OUT_D], f32, tag="wout")
                nc.tensor.matmul(
                    po[:], agg2[:, g * 128:(g + 1) * 128], w2_t[:],
                    start=True, stop=True,
                )
                ot = ppool.tile([128, OUT_D], f32, tag="ot")
                nc.vector.tensor_scalar(
                    out=ot[:], in0=po[:], scalar1=dinv_t[:, g:g + 1],
                    scalar2=None, op0=mybir.AluOpType.mult,
                )
                nc.vector.tensor_tensor(
                    out=ot[:], in0=ot[:], in1=b2_t[:], op=mybir.AluOpType.add
                )
                nc.sync.dma_start(OUTE[g * 128:(g + 1) * 128, :], ot[:])

    nc.compile()
    return nc


# ============================ entry point ============================

def prepare(x, edge_index, W1, b1, W2, b2):
    x = np.asarray(x, dtype=np.float32)
    W1 = np.asarray(W1, dtype=np.float32)
    b1 = np.asarray(b1, dtype=np.float32)
    W2 = np.asarray(W2, dtype=np.float32)
    b2 = np.asarray(b2, dtype=np.float32)

    sched, idx_all, sval_all, dinv_gt, dinv, stream_glob = _host_prep(edge_index)

    key = ("v5e", sched["CIDX"], sched["TG"])
    if key in _CACHE:
        nc = _CACHE[key]
    else:
        nc = build_program(sched)
        _CACHE[key] = nc

    xs = x * dinv[:, None]
    T1 = np.zeros((NTOT, F_IN), dtype=ml_dtypes.bfloat16)
    for c in range(NCORES):
        T1[c * SH:c * SH + NPC] = xs[c * NPC:(c + 1) * NPC].astype(ml_dtypes.bfloat16)

    TG = sched["TG"]
    b1_tile = np.tile(b1[None, :], (128, 1)).astype(np.float32)
    b2_tile = np.tile(b2[None, :], (128, 1)).astype(np.float32)

    in_maps = []
    for c in range(NCORES):
        m1 = T1[stream_glob[c]].reshape(TG, 128, F_IN).transpose(1, 0, 2)
        in_maps.append({
            "M1D": np.ascontiguousarray(m1),
            "IDX": np.ascontiguousarray(idx_all[c]),
            "SVAL": np.ascontiguousarray(sval_all[c]),
            "DINV": np.ascontiguousarray(dinv_gt[c]),
            "W1T": W1.astype(ml_dtypes.bfloat16),
            "B1T": b1_tile,
            "W2T": W2.astype(ml_dtypes.bfloat16),
            "B2T": b2_tile,
        })
    return nc, in_maps


def kernel(x, edge_index, W1, b1, W2, b2):
    from concourse.bass_utils import run_bass_kernel_spmd

    nc, in_maps = prepare(x, edge_index, W1, b1, W2, b2)
    r = run_bass_kernel_spmd(nc, in_maps, core_ids=list(range(NCORES)))
    out = np.empty((N, OUT_D), dtype=np.float32)
    for c in range(NCORES):
        out[c * NPC:(c + 1) * NPC] = r.results[c]["OUTE"][:NPC]
    return out


# revision 17
# speedup vs baseline: 1.7484x; 1.2352x over previous
"""Trainium2 Bass kernel for 2-layer GCN (nn_GCN_22866405884174).

Strategy (8 NeuronCores, dst-node sharding):
  out = A @ relu((A @ x) @ W1 + b1) @ W2 + b2   with A = D^-1/2 (Adj+I) D^-1/2
  (linear layers commute with aggregation, so each layer is: gather table
  rows by edge src + scatter-add by edge dst, then a small dense matmul).

  - Nodes sharded contiguously: core c owns dst nodes [c*12500, (c+1)*12500).
  - Layer 1: messages are HOST-materialized in slot order (the edge indices
    are known host-side), so the device just streams them contiguously via
    HWDGE — zero Q7 descriptor-generation work.
  - Layer 2: SWDGE dma_gather from the AllGathered hidden table (1024-index
    chunks, 4 queues = 4 Q7 core pairs generating descriptors in parallel).
  - Scatter-add: PE matmul msg[slots,feat]^T @ S[slots,128dst] accumulated
    into a full PSUM bank [128, 512] per 8-window block. S matrices are
    exact one-hot fp8 built on host (shared by both layers).
  - Between layers: AllGather of the bf16 hidden table across the 8 cores.
"""

import numpy as np
import ml_dtypes

# ---------------- problem constants (hardcoded per contract) ----------------
N = 100000
E = 1600000
F_IN = 128
HID = 64
OUT_D = 10

NCORES = 8
NPC = N // NCORES           # 12500 nodes per core
SH = 12544                  # padded shard rows (98 * 128)
NTOT = SH * NCORES          # 100352
SEC = 25088                 # table section rows (2 shards, < int16 range)
NSEC = 4
WDST = 64                   # dst window width
NWIN = (NPC + WDST - 1) // WDST   # 196 (last window = 20 dst)
WB = 8                      # windows per block
NBLK = (NWIN + WB - 1) // WB      # 25 (last block = 4 windows)
BCOLS = WB * WDST           # 512 psum cols per block
NG = SH // 128              # 98 node groups per shard
QH = 3136                   # quarter height (SH/4); table quarter q = concat of cores' q-th quarters
SENTINEL = 12500            # padding slots point at an arbitrary valid row (S column is 0)
CHUNK = 1024                # L2 gather chunk
PIECE = 32                  # L1 stream piece (groups per dma_start)

_CACHE = {}


# ============================ host preprocessing ============================

def _host_prep(edge_index):
    src = np.asarray(edge_index[0]).astype(np.int64)
    dst = np.asarray(edge_index[1]).astype(np.int64)
    loops = np.arange(N, dtype=np.int64)
    src = np.concatenate([src, loops])
    dst = np.concatenate([dst, loops])
    deg = np.bincount(dst, minlength=N).astype(np.float32)
    dinv = 1.0 / np.sqrt(deg)

    score = src // NPC
    sloc = src % NPC
    q = sloc // QH
    srow = q * SEC + score * QH + (sloc - q * QH)
    core = dst // NPC
    dloc = dst % NPC
    win = dloc // WDST
    sec = srow // SEC

    cellid = (core * NWIN + win) * NSEC + sec
    counts = np.bincount(cellid, minlength=NCORES * NWIN * NSEC).reshape(NCORES, NWIN, NSEC)
    n_cell = counts.max(axis=0)
    n_cell = np.maximum(((n_cell + 15) // 16) * 16, 128)   # 16-aligned, >= 128

    # ---- schedule: section-major slot streams, (block,sec) runs 128-aligned ----
    blocks = [[None] * NSEC for _ in range(NBLK)]
    sec_len = [0] * NSEC
    for s in range(NSEC):
        off = 0
        for b in range(NBLK):
            wlo, whi = b * WB, min(NWIN, (b + 1) * WB)
            cells = [int(n_cell[w, s]) for w in range(wlo, whi)]
            nbs = sum(cells)
            run = ((nbs + 127) // 128) * 128
            ngrp = run // 128
            bounds = np.cumsum([0] + cells)
            groups = []
            for j in range(ngrp):
                gslot = off + j * 128
                wi = int(np.searchsorted(bounds, j * 128, side="right") - 1)
                wi = min(wi, len(cells) - 1)
                base = min(wi * WDST, BCOLS - 128)
                groups.append((gslot, base))
            blocks[b][s] = {
                "cells": cells, "nbs": nbs, "run": run, "ngrp": ngrp,
                "groups": groups, "soff": off,
            }
            off += run
        sec_len[s] = off

    # S group offsets in (b, s, j) order
    TG = 0
    for b in range(NBLK):
        for s in range(NSEC):
            blocks[b][s]["gi"] = TG
            TG += blocks[b][s]["ngrp"]

    # idx tensor: queue-pinned chunk layout. Chunk (s,k) is assigned to SWDGE
    # queue q (round-robin in device issue order); its idx columns live at
    # qcol[(s,k)] and only in partitions [32q, 32q+32) (the worker core pair).
    chunks_tmp = [[] for _ in range(NSEC)]
    for s in range(NSEC):
        rem = sec_len[s]
        while rem > 0:
            chunks_tmp[s].append(min(CHUNK, rem))
            rem -= min(CHUNK, rem)
    qof = [0, 0, 0, 0]
    qmap = {}
    qn = 0
    kmax_t = max(len(chunks_tmp[s]) for s in range(NSEC))
    for k in range(kmax_t):
        for s in range(NSEC):
            if k >= len(chunks_tmp[s]):
                continue
            qmap[(s, k)] = (qn, qof[qn])
            qof[qn] += chunks_tmp[s][k] // 16
            qn = (qn + 1) % 4
    CIDX = max(qof)

    # slot-group offset of each section stream within the whole msg stream
    sec_goff = [0] * NSEC
    TOTG = 0
    for s in range(NSEC):
        sec_goff[s] = TOTG
        TOTG += sec_len[s] // 128

    chunks = chunks_tmp                  # per section: chunk sizes

    sort_key = (sec + NSEC * (win + NWIN * core))
    order = np.lexsort((dloc, sort_key))
    srow_s = srow[order]
    dloc_s = dloc[order]
    key_s = sort_key[order]

    idx_all = np.zeros((NCORES, 128, CIDX), dtype=np.int16)
    sval_all = np.zeros((NCORES, 128, TG, 128), dtype=ml_dtypes.float8_e4m3)
    dinv_gt = np.zeros((NCORES, 128, NG), dtype=np.float32)
    # global table row per slot (per core), for host-side L1 message gather
    stream_glob = np.zeros((NCORES, TG * 128), dtype=np.int64)

    cw_starts = np.searchsorted(key_s, np.arange(NCORES * NWIN * NSEC + 1))
    for c in range(NCORES):
        s_g = []
        s_p = []
        s_d = []
        for s in range(NSEC):
            stream = np.full(sec_len[s], SENTINEL, dtype=np.int64)
            dcol_st = np.full(sec_len[s], -1, dtype=np.int64)
            for b in range(NBLK):
                info = blocks[b][s]
                off = info["soff"]
                wlo, whi = b * WB, min(NWIN, (b + 1) * WB)
                for wi, w in enumerate(range(wlo, whi)):
                    cid = (c * NWIN + w) * NSEC + s
                    a, e = cw_starts[cid], cw_starts[cid + 1]
                    cnt = e - a
                    stream[off:off + cnt] = srow_s[a:e] - s * SEC
                    dcol_st[off:off + cnt] = dloc_s[a:e] - b * BCOLS
                    off += info["cells"][wi]
                # S coords for this (b, s): slots [soff, soff+run)
                t0, t1 = info["soff"], info["soff"] + info["run"]
                t = np.arange(t0, t1)
                dc = dcol_st[t0:t1]
                real = dc >= 0
                j = (t - t0) // 128
                bases = np.array([g[1] for g in info["groups"]], dtype=np.int64)
                scol = dc - bases[j]
                if real.any():
                    assert scol[real].min() >= 0 and scol[real].max() < 128
                s_g.append(info["gi"] + j[real])
                s_p.append((t[real] - t0) % 128)
                s_d.append(scol[real])
            st16 = stream.astype(np.int16).reshape(sec_len[s] // 16, 16).T
            for k in range(len(chunks[s])):
                q, col = qmap[(s, k)]
                ncol = chunks[s][k] // 16
                blk16 = st16[:, k * (CHUNK // 16):k * (CHUNK // 16) + ncol]
                idx_all[c, 32 * q:32 * q + 16, col:col + ncol] = blk16
                idx_all[c, 32 * q + 16:32 * q + 32, col:col + ncol] = blk16
            for b in range(NBLK):
                info = blocks[b][s]
                gi, soff, run = info["gi"], info["soff"], info["run"]
                stream_glob[c, gi * 128:gi * 128 + run] = stream[soff:soff + run] + s * SEC
        sval_all[c, np.concatenate(s_p), np.concatenate(s_g),
                 np.concatenate(s_d)] = ml_dtypes.float8_e4m3(1.0)

        dpad = np.zeros(SH, dtype=np.float32)
        dpad[:NPC] = dinv[c * NPC:(c + 1) * NPC]
        dinv_gt[c] = dpad.reshape(NG, 128).T

    sched = {
        "blocks": blocks, "chunks": chunks, "qmap": qmap,
        "sec_len": sec_len, "CIDX": CIDX, "TG": TG,
    }
    return sched, idx_all, sval_all, dinv_gt, dinv, stream_glob


# ============================ device program ============================

def build_program(sched):
    import concourse.bass as bass
    import concourse.bacc as bacc
    import concourse.tile as tile
    import concourse.mybir as mybir

    CIDX = sched["CIDX"]
    TG = sched["TG"]
    blocks = sched["blocks"]
    chunks = sched["chunks"]
    qmap = sched["qmap"]
    GBLK_MAX = max(sum(blocks[b][s]["ngrp"] for s in range(NSEC)) for b in range(NBLK))

    nc = bacc.Bacc(None, target_bir_lowering=False, debug=False, num_swdge_queues=4,
                   dynamic_dma_scratch_size=16384)
    f32 = mybir.dt.float32
    bf16 = mybir.dt.bfloat16
    fp8 = mybir.dt.float8e4
    i16 = mybir.dt.int16

    M1D = nc.dram_tensor("M1D", [128, TG, F_IN], bf16, kind="ExternalInput")
    IDX = nc.dram_tensor("IDX", [128, CIDX], i16, kind="ExternalInput")
    SVAL = nc.dram_tensor("SVAL", [128, TG, 128], fp8, kind="ExternalInput")
    DINV = nc.dram_tensor("DINV", [128, NG], f32, kind="ExternalInput")
    W1T = nc.dram_tensor("W1T", [F_IN, HID], bf16, kind="ExternalInput")
    B1T = nc.dram_tensor("B1T", [128, HID], f32, kind="ExternalInput")
    W2T = nc.dram_tensor("W2T", [HID, OUT_D], bf16, kind="ExternalInput")
    B2T = nc.dram_tensor("B2T", [128, OUT_D], f32, kind="ExternalInput")
    OUTE = nc.dram_tensor("OUTE", [SH, OUT_D], f32, kind="ExternalOutput")

    t_local = nc.dram_tensor("t_local", [SH, F_IN], bf16)
    t_full = nc.dram_tensor("t_full", [NTOT, F_IN], bf16, addr_space="Shared")

    with tile.TileContext(nc) as tc:
        with (
            tc.tile_pool(name="resident", bufs=1) as rpool,
            tc.tile_pool(name="msg", bufs=20) as mpool,
            tc.tile_pool(name="msgb", bufs=4) as bpool,
            tc.tile_pool(name="sv", bufs=3) as spool,
            tc.tile_pool(name="post", bufs=4) as ppool,
            tc.tile_pool(name="psum", bufs=3, space="PSUM") as psum_pool,
            tc.tile_pool(name="psum2", bufs=4, space="PSUM") as psum_pool2,
        ):
            idx_t = rpool.tile([128, CIDX], i16)
            nc.sync.dma_start(idx_t[:], IDX[:])
            dinv_t = rpool.tile([128, NG], f32)
            nc.sync.dma_start(dinv_t[:], DINV[:])
            w1_t = rpool.tile([F_IN, HID], bf16)
            nc.sync.dma_start(w1_t[:], W1T[:])
            b1_t = rpool.tile([128, HID], f32)
            nc.sync.dma_start(b1_t[:], B1T[:])
            w2_t = rpool.tile([HID, OUT_D], bf16)
            nc.sync.dma_start(w2_t[:], W2T[:])
            b2_t = rpool.tile([128, OUT_D], f32)
            nc.sync.dma_start(b2_t[:], B2T[:])

            agg1 = rpool.tile([128, SH], bf16)
            agg2 = rpool.tile([HID, SH], bf16)

            def scatter_layer(table):
                # layer 2: SWDGE gathers, round-robin over 4 queues
                msg_tiles = {}
                kmax = max(len(chunks[s]) for s in range(NSEC))
                for k in range(kmax):
                    for s in range(NSEC):
                        if k >= len(chunks[s]):
                            continue
                        csz = chunks[s][k]
                        ng = (csz + 127) // 128
                        qn, col = qmap[(s, k)]
                        msg = mpool.tile([128, CHUNK // 128, F_IN], bf16, tag="msg")
                        nc.gpsimd.dma_gather(
                            msg[:, :ng, :],
                            table[s * SEC:(s + 1) * SEC, :],
                            idx_t[:, col:col + csz // 16],
                            csz,
                            csz,
                            F_IN,
                            single_packet=True,
                            queue_num=qn,
                        )
                        msg_tiles[(s, k)] = msg

                def access(s, gslot, gi):
                    return msg_tiles[(s, gslot // CHUNK)][:, (gslot % CHUNK) // 128, :]
                return access

            def stream_layer():
                # layer 1: host-materialized messages in block-consumption
                # (gi) order -- strictly sequential contiguous HWDGE stream
                msg_tiles = {}
                for p0 in range(0, TG, PIECE):
                    ng = min(PIECE, TG - p0)
                    msg = bpool.tile([128, PIECE, F_IN], bf16, tag="msgb")
                    nc.scalar.dma_start(msg[:, :ng, :], M1D[:, p0:p0 + ng, :])
                    msg_tiles[p0 // PIECE] = msg

                def access(s, gslot, gi):
                    return msg_tiles[gi // PIECE][:, gi % PIECE, :]
                return access

            def consume_blocks(access, agg):
                for b in range(NBLK):
                    blo = b * BCOLS
                    blen = min(BCOLS, NPC - blo)
                    gblk = sum(blocks[b][s]["ngrp"] for s in range(NSEC))
                    g0 = blocks[b][0]["gi"]
                    s_t = spool.tile([128, GBLK_MAX, 128], fp8, tag="sval")
                    nc.sync.dma_start(
                        s_t[:, :gblk, :], SVAL[:, g0:g0 + gblk, :]
                    )
                    acc = psum_pool.tile([128, BCOLS], f32, tag="acc")
                    mm = 0
                    for s in range(NSEC):
                        info = blocks[b][s]
                        for (gslot, base) in info["groups"]:
                            nc.tensor.matmul(
                                acc[:, base:base + 128],
                                access(s, gslot, g0 + mm),
                                s_t[:, mm, :],
                                start=(mm == 0),
                                stop=(mm == gblk - 1),
                            )
                            mm += 1
                    nc.vector.tensor_copy(
                        agg[:, blo:blo + blen], acc[:agg.shape[0], :blen]
                    )
                nc.vector.memset(agg[:, NPC:SH], 0.0)

            # ---------------- layer 1 ----------------
            consume_blocks(stream_layer(), agg1)

            # quarter q of t_local complete once group (q+1)*QH/128 (rounded
            # up) is stored; fire that quarter's AllGather piece right there so
            # the collective pipelines with the transform loop.
            ag_after = {((qq + 1) * QH + 127) // 128 - 1: qq for qq in range(4)}
            for g in range(NG):
                ph = psum_pool2.tile([128, HID], f32, tag="wout")
                nc.tensor.matmul(
                    ph[:], agg1[:, g * 128:(g + 1) * 128], w1_t[:],
                    start=True, stop=True,
                )
                tmp = ppool.tile([128, HID], f32, tag="tmp")
                nc.vector.tensor_scalar(
                    out=tmp[:], in0=ph[:], scalar1=dinv_t[:, g:g + 1],
                    scalar2=None, op0=mybir.AluOpType.mult,
                )
                nc.vector.tensor_tensor(
                    out=tmp[:], in0=tmp[:], in1=b1_t[:], op=mybir.AluOpType.add
                )
                t_out = ppool.tile([128, F_IN], bf16, tag="tout")
                nc.vector.memset(t_out[:, HID:], 0.0)
                nc.vector.tensor_scalar(
                    out=t_out[:, :HID], in0=tmp[:], scalar1=0.0,
                    scalar2=dinv_t[:, g:g + 1], op0=mybir.AluOpType.max,
                    op1=mybir.AluOpType.mult,
                )
                nc.scalar.dma_start(t_local[g * 128:(g + 1) * 128, :], t_out[:])
                if g in ag_after:
                    qq = ag_after[g]
                    nc.gpsimd.collective_compute(
                        "AllGather",
                        mybir.AluOpType.bypass,
                        replica_groups=[list(range(NCORES))],
                        ins=[t_local[qq * QH:(qq + 1) * QH]],
                        outs=[t_full[qq * SEC:(qq + 1) * SEC]],
                    )

            # ---------------- layer 2 ----------------
            consume_blocks(scatter_layer(t_full), agg2)

            for g in range(NG):
                po = psum_pool2.tile([128, OUT_D], f32, tag="wout")
                nc.tensor.matmul(
                    po[:], agg2[:, g * 128:(g + 1) * 128], w2_t[:],
                    start=True, stop=True,
                )
                ot = ppool.tile([128, OUT_D], f32, tag="ot")
                nc.vector.tensor_scalar(
                    out=ot[:], in0=po[:], scalar1=dinv_t[:, g:g + 1],
                    scalar2=None, op0=mybir.AluOpType.mult,
                )
                nc.vector.tensor_tensor(
                    out=ot[:], in0=ot[:], in1=b2_t[:], op=mybir.AluOpType.add
                )
                nc.sync.dma_start(OUTE[g * 128:(g + 1) * 128, :], ot[:])

    nc.compile()
    return nc


# ============================ entry point ============================

def prepare(x, edge_index, W1, b1, W2, b2):
    x = np.asarray(x, dtype=np.float32)
    W1 = np.asarray(W1, dtype=np.float32)
    b1 = np.asarray(b1, dtype=np.float32)
    W2 = np.asarray(W2, dtype=np.float32)
    b2 = np.asarray(b2, dtype=np.float32)

    sched, idx_all, sval_all, dinv_gt, dinv, stream_glob = _host_prep(edge_index)

    key = ("v5f", sched["CIDX"], sched["TG"])
    if key in _CACHE:
        nc = _CACHE[key]
    else:
        nc = build_program(sched)
        _CACHE[key] = nc

    xs = x * dinv[:, None]
    # quarter-interleaved table layout: quarter q = concat over cores of their
    # q-th quarter of padded shard rows
    T1 = np.zeros((NTOT, F_IN), dtype=ml_dtypes.bfloat16)
    for c in range(NCORES):
        sh = np.zeros((SH, F_IN), dtype=ml_dtypes.bfloat16)
        sh[:NPC] = xs[c * NPC:(c + 1) * NPC].astype(ml_dtypes.bfloat16)
        for q in range(4):
            T1[q * SEC + c * QH:(q * SEC + (c + 1) * QH)] = sh[q * QH:(q + 1) * QH]

    TG = sched["TG"]
    b1_tile = np.tile(b1[None, :], (128, 1)).astype(np.float32)
    b2_tile = np.tile(b2[None, :], (128, 1)).astype(np.float32)

    in_maps = []
    for c in range(NCORES):
        m1 = T1[stream_glob[c]].reshape(TG, 128, F_IN).transpose(1, 0, 2)
        in_maps.append({
            "M1D": np.ascontiguousarray(m1),
            "IDX": np.ascontiguousarray(idx_all[c]),
            "SVAL": np.ascontiguousarray(sval_all[c]),
            "DINV": np.ascontiguousarray(dinv_gt[c]),
            "W1T": W1.astype(ml_dtypes.bfloat16),
            "B1T": b1_tile,
            "W2T": W2.astype(ml_dtypes.bfloat16),
            "B2T": b2_tile,
        })
    return nc, in_maps


def kernel(x, edge_index, W1, b1, W2, b2):
    from concourse.bass_utils import run_bass_kernel_spmd

    nc, in_maps = prepare(x, edge_index, W1, b1, W2, b2)
    r = run_bass_kernel_spmd(nc, in_maps, core_ids=list(range(NCORES)))
    out = np.empty((N, OUT_D), dtype=np.float32)
    for c in range(NCORES):
        out[c * NPC:(c + 1) * NPC] = r.results[c]["OUTE"][:NPC]
    return out
